# revision 40
# baseline (speedup 1.0000x reference)
"""Trainium2 Bass kernel: per-image segment-mean repaint (DeepgazeSpade).

Reference computation per image b:
  seg_ds        = segmap[::8, ::8]                  (nearest downsample: 384/48 = 512/64 = 8)
  sums[s, c]    = sum_{p : seg_ds[p] == s} feats[c, p]
  counts[s]     = |{p : seg_ds[p] == s}|
  avg[s, c]     = sums / max(counts, 1)             (0 for empty segments)
  out[c, y, x]  = avg[segmap[y, x], c]

Sharding: 8 cores = 4 images x 2 row-halves (pure data parallel, no
collectives). Each core computes the (cheap) per-image segment stats from the
full downsampled grid and paints its own half of the image.

Host prep (dtype casts / reshapes only): feats shipped pre-transposed as
[3072 px, 257] fp32 with a fused ones column (counts fall out of the same
matmul chain); downsampled seg ids shipped as [128, 24] fp32 in chunk layout;
paint seg ids shipped as bf16 (values < 128 are bf16-exact).

Per-core device algorithm:
  stats: 24 accumulating matmuls ohd[px,seg].T @ ftT[px, 257] -> [seg, 256
         sums | counts] in PSUM; one-hot ohd built by tensor_scalar(is_equal)
         against an iota-row constant; avg = sums * reciprocal(max(counts,1)),
         then rounded to bf16 (harness gate is 2e-2; bf16 is ~1e-3).
  paint: per 1024-px pair: one K=1 bf16 matmul broadcasts the seg ids to all
         128 partitions (bf16 PSUM bank holds 1024); tensor_scalar(is_equal)
         against the partition-index iota gives the one-hot [s=128, 1024] in
         bf16 SBUF; two bf16 matmuls (stationary avg, N=1024, bf16 PSUM out)
         gather all 256 channels; scalar+vector copies upconvert to fp32 SBUF;
         DMA out. bf16 PSUM write is exact here: the one-hot selects a single
         bf16 value.
"""

import numpy as np
import ml_dtypes

B, C = 4, 256
HF, WF = 48, 64
HIMG, WIMG = 384, 512
S = 128
NPIX_DS = HF * WF              # 3072 downsampled pixels
NCHUNK_DS = NPIX_DS // 128     # 24
CW = C + 1                     # 257: channels + fused ones column
HALF_ROWS = HIMG // 2          # 192
NPIX_HALF = HALF_ROWS * WIMG   # 98304 pixels per core
TILE = 512
PAIR = 2 * TILE                # 1024-px paint unit
NPAIRS = NPIX_HALF // PAIR     # 96
WIN = 8192                     # seg-id window on partition 0 (bf16, 16 KiB)
WPAIRS = WIN // PAIR           # 8

_CACHE = {}
LAST_RESULTS = None
TRACE = False


def _body(tc, out, ftT, ds, seg_bf, dummy=None):
    import concourse.mybir as mybir

    dt = mybir.dt
    eq = mybir.AluOpType.is_equal
    mul = mybir.AluOpType.mult
    nc = tc.nc

    with (
        tc.tile_pool(name="const", bufs=1) as cpool,
        tc.tile_pool(name="oh", bufs=4) as ohpool,
        tc.tile_pool(name="ob", bufs=6) as obpool,
    ):
        # ---- constants ----
        iota_row_i = cpool.tile([128, 128], dt.int32)
        nc.gpsimd.iota(iota_row_i[:], pattern=[[1, 128]], base=0, channel_multiplier=0)
        iota_row_f = cpool.tile([128, 128], dt.float32)
        nc.vector.tensor_copy(iota_row_f[:], iota_row_i[:])

        iota_col_i = cpool.tile([128, 1], dt.int32)
        nc.gpsimd.iota(iota_col_i[:], pattern=[[0, 1]], base=0, channel_multiplier=1)
        iota_col_f = cpool.tile([128, 1], dt.float32)
        nc.vector.tensor_copy(iota_col_f[:], iota_col_i[:])

        ones_bf = cpool.tile([128, 128], dt.bfloat16)
        nc.vector.memset(ones_bf[:], 1.0)
        warm_mv = cpool.tile([1, TILE], dt.bfloat16)
        nc.vector.memset(warm_mv[:], 1.0)

        # ---- loads (small tensors first so stats can start early; ds/seg
        # issue on the idle Pool queue so they don't serialize behind ft) ----
        ds_f = cpool.tile([128, NCHUNK_DS], dt.float32)
        nc.sync.dma_start(ds_f[:], ds)

        # all seg ids resident: rows {0,32,64} hold thirds of the half-image
        # (matmul operands may start at partition 0/32/64)
        SEGQ = NPIX_HALF // 3      # 32768
        seg_all = cpool.tile([128, SEGQ], dt.bfloat16)
        for r in range(3):
            nc.sync.dma_start(
                seg_all[32 * r:32 * r + 1, :],
                seg_bf[r * SEGQ:(r + 1) * SEGQ].rearrange("(o f) -> o f", o=1),
            )

        # chunk j / partition p = ds pixel j*128 + p; free dim j*CW + c
        # (host ships this layout; 4 quarter-DMAs so chunk 0 lands early)
        ft = cpool.tile([128, NCHUNK_DS * CW], dt.bfloat16)
        QW = NCHUNK_DS * CW // 4
        for q in range(4):
            nc.sync.dma_start(ft[:, q * QW:(q + 1) * QW],
                              ftT[:, q * QW:(q + 1) * QW])

        # ---- PE clock warmup during the ft load (HAM releases the clock
        # gate after ~3us of sustained matmul activity) ----
        warm_ctx = tc.tile_pool(name="warm", bufs=1, space="PSUM")
        wp = warm_ctx.__enter__()
        warm_ps = wp.tile([128, TILE], dt.float32)
        for _ in range(8):
            nc.tensor.matmul(warm_ps[:], ones_bf[:1, :], warm_mv[:1, :],
                             start=True, stop=True)
        warm_ctx.__exit__(None, None, None)

        # ---- stats: [seg, 256 sums | counts] via one accumulating chain ----
        stats_ctx = tc.tile_pool(name="ps", bufs=1, space="PSUM")
        ps = stats_ctx.__enter__()
        psum_s = ps.tile([128, CW], dt.float32)
        for j in range(NCHUNK_DS):
            ohd = ohpool.tile([128, 128], dt.bfloat16, tag="ohd")
            nc.vector.tensor_scalar(ohd[:], iota_row_f[:], ds_f[:, j:j + 1], None, eq)
            nc.tensor.matmul(
                psum_s[:], ohd[:], ft[:, j * CW:(j + 1) * CW],
                start=(j == 0), stop=(j == NCHUNK_DS - 1),
            )

        PPQ = SEGQ // PAIR         # 32 pairs per seg_all row

        def build_oh(pr):
            r = 32 * (pr // PPQ)
            o = (pr % PPQ) * PAIR
            oh = ohpool.tile([128, PAIR], dt.bfloat16, tag="oh")
            for half in range(2):
                hs = slice(half * TILE, (half + 1) * TILE)
                bc = bcpool.tile([128, TILE], dt.float32, tag="bc")
                nc.tensor.matmul(
                    bc[:], ones_bf[r:r + 1, :],
                    seg_all[r:r + 1, o + half * TILE:o + (half + 1) * TILE],
                    start=True, stop=True,
                )
                nc.vector.tensor_scalar(oh[:, hs], bc[:], iota_col_f[:], None, eq)
            return oh

        ohs = {}

        cnt1 = cpool.tile([128, 1], dt.float32)
        nc.vector.tensor_scalar_max(cnt1[:], psum_s[:, C:CW], 1.0)
        rec = cpool.tile([128, 1], dt.float32)
        nc.vector.reciprocal(rec[:], cnt1[:])
        avg_bf = cpool.tile([128, C], dt.bfloat16)
        nc.vector.tensor_scalar(avg_bf[:], psum_s[:, 0:C], rec[:], None, mul)
        stats_ctx.__exit__(None, None, None)

        # ---- paint: 1024-px pairs ----
        bc_ctx = tc.tile_pool(name="bc", bufs=2, space="PSUM")
        bcpool = bc_ctx.__enter__()
        po_ctx = tc.tile_pool(name="po", bufs=3, space="PSUM")
        po = po_ctx.__enter__()
        for pr in range(NPAIRS):
            oh = ohs.pop(pr) if pr in ohs else build_oh(pr)

            for cc in range(2):
                pot = po.tile([128, PAIR], dt.float32, tag="po")
                for half in range(2):
                    hs = slice(half * TILE, (half + 1) * TILE)
                    nc.tensor.matmul(
                        pot[:, hs], avg_bf[:, cc * 128:(cc + 1) * 128], oh[:, hs],
                        start=True, stop=True,
                    )
                ob = obpool.tile([128, PAIR], dt.float32, tag="ob")
                nc.scalar.copy(ob[:], pot[:])
                eng = nc.sync if cc == 0 else nc.gpsimd
                eng.dma_start(
                    out[cc * 128:(cc + 1) * 128, pr * PAIR:(pr + 1) * PAIR], ob[:]
                )
        po_ctx.__exit__(None, None, None)
        bc_ctx.__exit__(None, None, None)
        if dummy is not None:
            # bench mode: tiny ExternalOutput so the big `out` can be
            # internal DRAM (avoids shipping 100 MB/core through axon)
            nc.sync.dma_start(dummy.rearrange("(o f) -> o f", o=1),
                              ones_bf[0:1, 0:1])


def _build_nc(reps=1, bench=False):
    import concourse.bacc as bacc
    import concourse.mybir as mybir
    import concourse.tile as tile

    dt = mybir.dt
    nc = bacc.Bacc("TRN2", target_bir_lowering=False, debug=False,
                   enable_asserts=False)
    ftT = nc.dram_tensor("ftT", [128, NCHUNK_DS * CW], dt.bfloat16,
                         kind="ExternalInput").ap()
    ds = nc.dram_tensor("ds", [128, NCHUNK_DS], dt.float32,
                        kind="ExternalInput").ap()
    seg_bf = nc.dram_tensor("seg_bf", [NPIX_HALF], dt.bfloat16,
                            kind="ExternalInput").ap()
    if bench:
        out = nc.dram_tensor("out", [C, NPIX_HALF], dt.float32).ap()
    else:
        out = nc.dram_tensor("out", [C, NPIX_HALF], dt.float32,
                             kind="ExternalOutput").ap()
    dummy = None
    if bench:
        dummy = nc.dram_tensor("bench_out", [1], dt.bfloat16,
                               kind="ExternalOutput").ap()
    with tile.TileContext(nc) as tc:
        if reps == 1:
            _body(tc, out, ftT, ds, seg_bf, dummy)
        else:
            with tc.For_i(0, reps, 1):
                _body(tc, out, ftT, ds, seg_bf, dummy)
    nc.compile()
    return nc


def make_in_maps(F, seg):
    """F: [B, C, NPIX_DS] float32; seg: [B, HIMG, WIMG] int."""
    F = np.asarray(F, dtype=np.float32).reshape(B, C, NPIX_DS)
    seg = np.clip(np.asarray(seg), 0, S - 1).astype(np.int32)
    in_maps = []
    for core in range(8):
        b, h = core // 2, core % 2
        # ft[p, j*CW + c] = feats^T[j*128 + p, c], ones fused at c = C
        ftT = np.empty((NCHUNK_DS, 128, CW), dtype=ml_dtypes.bfloat16)
        ftT[:, :, :C] = F[b].T.reshape(NCHUNK_DS, 128, C)
        ftT[:, :, C] = 1.0
        ftT = np.ascontiguousarray(
            ftT.transpose(1, 0, 2).reshape(128, NCHUNK_DS * CW))
        dsb = seg[b, ::8, ::8].reshape(NCHUNK_DS, 128)
        seg_half = seg[b, h * HALF_ROWS:(h + 1) * HALF_ROWS, :].reshape(-1)
        in_maps.append({
            "ftT": ftT,
            "ds": np.ascontiguousarray(dsb.T.astype(np.float32)),
            "seg_bf": seg_half.astype(ml_dtypes.bfloat16),
        })
    return in_maps


def kernel(F_semantic_features, segmentation_mask, num_total_segments=None):
    global LAST_RESULTS
    from concourse.bass_utils import run_bass_kernel_spmd

    F = np.asarray(F_semantic_features, dtype=np.float32)
    seg = np.asarray(segmentation_mask)

    if "nc" not in _CACHE:
        _CACHE["nc"] = _build_nc()
    nc = _CACHE["nc"]

    in_maps = make_in_maps(F.reshape(B, C, NPIX_DS), seg)
    res = run_bass_kernel_spmd(nc, in_maps, core_ids=list(range(8)),
                               trace=bool(TRACE))
    LAST_RESULTS = res

    imgs = []
    for b in range(B):
        top = res.results[2 * b]["out"].reshape(C, HALF_ROWS, WIMG)
        bot = res.results[2 * b + 1]["out"].reshape(C, HALF_ROWS, WIMG)
        imgs.append(np.concatenate([top, bot], axis=1))
    return np.stack(imgs).astype(np.float32)


if __name__ == "__main__":
    rng = np.random.default_rng(0)
    F = rng.standard_normal((B, C, HF, WF), dtype=np.float32)
    seg = rng.integers(0, S, size=(B, HIMG, WIMG)).astype(np.int64)
    outv = kernel(F, seg, S)
    print("out", outv.shape, outv.dtype, float(outv.mean()))
